# revision 14
# baseline (speedup 1.0000x reference)
"""Causal self-attention (T=2048, C=1024, H=16) on 8 Trainium2 NeuronCores.

Tensor-parallel over heads: each core owns 2 heads (wqkv row-shard), computes
qkv + attention for its heads, all-gathers the per-head attention outputs, then
computes its 128-column slice of the output projection (proj column-shard).

Structure: one software-pipelined loop over 4 token chunks of 512. For chunk g:
qkv(g) -> attention(g, both heads interleaved) -> AllGather(g) -> proj(g).
Later chunks' compute overlaps earlier chunks' collectives.

Layout notes (per core c, heads 2c and 2c+1):
  - xT   [1024, 2048]  x transposed (shared by all cores)
  - wT   [1024, 384]   wqkv rows for (k,q,v) of this core's heads, transposed;
                       q-rows pre-scaled by 1/sqrt(64)=0.125 (exact)
  - qT/kT/vT [128, 2048] in SBUF: rows = 2 heads x 64 dims, cols = tokens
  - scoresT  [s, t] tiles computed directly (no probs transpose needed);
    softmax denominator comes from an appended ones-column on v (row 64 of the
    attnT psum accumulator), so no cross-partition reductions are needed.
  - exp() is safe without max-subtraction: |scores| < 4 for this problem.

All matmul operands use float32r (single-pass PE, 4x the fp32 rate); set
KERNEL_FP32_EXACT=1 to fall back to exact fp32 matmuls.
"""

import os
import numpy as np

os.environ.setdefault("NEURON_RT_DBG_RDH_CC", "0")  # prefer mesh collectives

import concourse.bass as bass
import concourse.mybir as mybir
import concourse.tile as tile
from concourse import bacc
from concourse import bass_utils

T = 2048
C = 1024
H = 16
D = 64
N_CORES = 8
P = 128
NT = T // P          # 16 token tiles
NG = T // 512        # 4 column chunks of 512
NO = C // P          # 8 contraction subtiles

F32 = mybir.dt.float32
FAST_MM = os.environ.get("KERNEL_FP32_EXACT", "0") != "1"
MM = mybir.dt.float32r if FAST_MM else mybir.dt.float32


def _build():
    nc = bacc.Bacc("TRN2", target_bir_lowering=False, debug=False,
                   num_devices=N_CORES)

    xT = nc.dram_tensor("xT", [C, T], MM, kind="ExternalInput").ap()
    wT = nc.dram_tensor("wT", [C, 3 * P], MM, kind="ExternalInput").ap()
    bqkv = nc.dram_tensor("bqkv", [P, 3], F32, kind="ExternalInput").ap()
    pwT = nc.dram_tensor("pwT", [C, P], MM, kind="ExternalInput").ap()
    pb = nc.dram_tensor("pb", [P, 1], F32, kind="ExternalInput").ap()
    mask01 = nc.dram_tensor("mask01", [P, P], MM, kind="ExternalInput").ap()
    onesd = nc.dram_tensor("onesd", [P, D], MM, kind="ExternalInput").ap()
    ident = nc.dram_tensor("ident", [P, P], MM, kind="ExternalInput").ap()
    outT = nc.dram_tensor("outT", [P, T], F32, kind="ExternalOutput").ap()

    xT3 = xT.rearrange("(o p) t -> p o t", p=P)      # [128, 8, 2048]
    wT3 = wT.rearrange("(o p) j -> p o j", p=P)      # [128, 8, 384]
    pwT3 = pwT.rearrange("(o p) j -> p o j", p=P)    # [128, 8, 128]

    # wqkv section order in wT columns: k, q, v (k first so attention's
    # score matmuls can start earliest in the pipelined schedule)
    JK, JQ, JV = 0, 1, 2

    with tile.TileContext(nc) as tc:
        with (
            tc.tile_pool(name="const", bufs=1) as constp,
            tc.tile_pool(name="big", bufs=1) as bigp,
            tc.tile_pool(name="work", bufs=4) as workp,
            tc.tile_pool(name="projw", bufs=2) as projwp,
            tc.tile_pool(name="outp", bufs=3) as outp,
            tc.tile_pool(name="ps_big", bufs=2, space="PSUM") as ps_big,
            tc.tile_pool(name="ps_at", bufs=2, space="PSUM") as ps_at,
            tc.tile_pool(name="ps_sm", bufs=2, space="PSUM") as ps_sm,
            tc.tile_pool(name="dram", bufs=1, space="DRAM") as dram,
        ):
            # ---- inputs: x and wqkv weights first (QKV needs them now) ----
            x_sb = bigp.tile([P, NO, T], MM, name="x")
            wT_sb = constp.tile([P, NO, 3 * P], MM, name="wT")
            for o in range(NO):
                nc.sync.dma_start(wT_sb[:, o, :], wT3[:, o, :])
                nc.sync.dma_start(x_sb[:, o, :], xT3[:, o, :])
            bq_sb = constp.tile([P, 3], F32, name="bqkv")
            nc.sync.dma_start(bq_sb[:], bqkv)
            id_sb = constp.tile([P, P], MM, name="ident")
            nc.sync.dma_start(id_sb[:], ident)
            mask_sb = constp.tile([P, P], MM, name="mask")
            nc.sync.dma_start(mask_sb[:], mask01)
            onesv_sb = constp.tile([P, NT], MM, name="onesv")
            nc.sync.dma_start(onesv_sb[:], onesd[:, 0:NT])
            ones_sb = constp.tile([1, D], F32, name="ones")
            nc.vector.memset(ones_sb[:], 1.0)
            pwT_sb = constp.tile([P, NO, P], MM, name="pwT")
            pb_sb = constp.tile([P, 1], F32, name="pb")

            qkvT = [bigp.tile([P, T], MM, name=n) for n in ("kT", "qT", "vT")]
            kT_sb, qT_sb, vT_sb = qkvT
            v_sb = bigp.tile([P, NT, 130], MM, name="v")
            nc.vector.tensor_copy(out=v_sb[:, :, D:D + 1],
                                  in_=onesv_sb[:, :, None])
            nc.vector.tensor_copy(out=v_sb[:, :, 2 * D + 1:2 * D + 2],
                                  in_=onesv_sb[:, :, None])

            ag_ins = [dram.tile([P, 512], MM, name=f"agi{g}") for g in range(NG)]
            ag_outs = [dram.tile([C, 512], MM, addr_space="Shared",
                                 name=f"ago{g}") for g in range(NG)]

            for g in range(NG):
                cols = slice(g * 512, (g + 1) * 512)

                # ---- qkv for this token chunk (k, q, then v) ----
                for j in (JK, JQ, JV):
                    ps = ps_big.tile([P, 1024], F32, name="sc2")
                    for o in range(NO):
                        nc.tensor.matmul(
                            ps[:, 0:512],
                            lhsT=wT_sb[:, o, j * P:(j + 1) * P],
                            rhs=x_sb[:, o, cols],
                            start=(o == 0), stop=(o == NO - 1),
                        )
                    # bias-add + copy on DVE (ACT stays free for exp)
                    nc.vector.tensor_scalar_add(qkvT[j][:, cols], ps[:, 0:512],
                                                bq_sb[:, j:j + 1])

                # v -> [s, d] tiles (with the ones column per head)
                for st in range(4 * g, 4 * g + 4):
                    pst = ps_sm.tile([P, 512], MM, name="sm")
                    nc.tensor.transpose(pst[:, 0:P],
                                        vT_sb[:, st * P:(st + 1) * P], id_sb[:])
                    nc.vector.tensor_copy(out=v_sb[:, st, 0:D],
                                          in_=pst[:, 0:D])
                    nc.vector.tensor_copy(out=v_sb[:, st, D + 1:2 * D + 1],
                                          in_=pst[:, D:2 * D])

                # ---- attention for chunk g, both heads interleaved ----
                ats = [ps_at.tile([P, 512], F32, name="at") for _ in range(2)]
                for j in range(4 * g + 4):
                    t0 = 512 * g if j < 4 * g else P * j
                    w_ = 512 * (g + 1) - t0
                    sc2 = ps_big.tile([P, 1024], F32, name="sc2")
                    for h in range(2):
                        nc.tensor.matmul(
                            sc2[:, 512 * h:512 * h + w_],
                            lhsT=kT_sb[h * D:(h + 1) * D, j * P:(j + 1) * P],
                            rhs=qT_sb[h * D:(h + 1) * D, t0:t0 + w_],
                            start=True, stop=True,
                        )
                    e2 = workp.tile([P, 1024], MM, name="e2")
                    if w_ == 512:
                        nc.scalar.activation(e2[:], sc2[:],
                                             mybir.ActivationFunctionType.Exp)
                    else:
                        for h in range(2):
                            nc.scalar.activation(
                                e2[:, 512 * h:512 * h + w_],
                                sc2[:, 512 * h:512 * h + w_],
                                mybir.ActivationFunctionType.Exp)
                    if j >= 4 * g:
                        # zero the strictly-upper (t<s) part of the diag block
                        for h in range(2):
                            nc.vector.tensor_mul(
                                out=e2[:, 512 * h:512 * h + P],
                                in0=e2[:, 512 * h:512 * h + P],
                                in1=mask_sb[:])
                    for h in range(2):
                        nc.tensor.matmul(
                            ats[h][:D + 1, t0 - 512 * g:512],
                            lhsT=v_sb[:, j, h * (D + 1):(h + 1) * (D + 1)],
                            rhs=e2[:, 512 * h:512 * h + w_],
                            start=(j == 0), stop=(j == 4 * g + 3),
                        )
                for h in range(2):
                    at = ats[h]
                    # rows 0..63 = unnormalized attnT, row 64 = softmax denom
                    rs = workp.tile([1, 512], F32, name="rs")
                    nc.vector.tensor_copy(out=rs[:], in_=at[D:D + 1, :])
                    rb = ps_sm.tile([P, 512], F32, name="sm")
                    nc.tensor.matmul(rb[:D, :], lhsT=ones_sb[:], rhs=rs[:],
                                     start=True, stop=True)
                    rr = workp.tile([D, 512], F32, name="rr")
                    nc.vector.reciprocal(rr[:], rb[:D, :])
                    ao = outp.tile([D, 512], MM, name="ao")
                    nc.vector.tensor_mul(out=ao[:], in0=at[0:D, :], in1=rr[:])
                    nc.sync.dma_start(ag_ins[g][h * D:(h + 1) * D, :], ao[:])

                nc.gpsimd.collective_compute(
                    "AllGather",
                    mybir.AluOpType.bypass,
                    replica_groups=[list(range(N_CORES))],
                    ins=[ag_ins[g].opt()],
                    outs=[ag_outs[g].opt()],
                )

                if g == 0:
                    # projection weights: needed from the first proj chunk on
                    nc.sync.dma_start(pwT_sb[:], pwT3)
                    nc.sync.dma_start(pb_sb[:], pb)

                # ---- projection for chunk g ----
                ag3 = ag_outs[g][:].rearrange("(o p) t -> p o t", p=P)
                r_sb = projwp.tile([P, NO, 512], MM, name="agr")
                nc.sync.dma_start(r_sb[:], ag3[:])
                psp = ps_at.tile([P, 512], F32, name="at")
                for o in range(NO):
                    nc.tensor.matmul(psp[:], lhsT=pwT_sb[:, o, :],
                                     rhs=r_sb[:, o, :],
                                     start=(o == 0), stop=(o == NO - 1))
                ob = outp.tile([P, 512], F32, name="ob")
                nc.scalar.activation(ob[:], psp[:],
                                     mybir.ActivationFunctionType.Identity,
                                     bias=pb_sb[:])
                nc.sync.dma_start(outT[:, cols], ob[:])

    nc.compile()
    return nc


_NC = None
LAST_RESULT = None


def _get_nc():
    global _NC
    if _NC is None:
        _NC = _build()
    return _NC


def _prep_inputs(x, wqkv_w, wqkv_b, proj_w, proj_b):
    x = np.asarray(x, np.float32)
    wqkv_w = np.asarray(wqkv_w, np.float32)
    wqkv_b = np.asarray(wqkv_b, np.float32)
    proj_w = np.asarray(proj_w, np.float32)
    proj_b = np.asarray(proj_b, np.float32)

    scale = np.float32(1.0 / np.sqrt(D))  # 0.125 exactly
    xT = np.ascontiguousarray(x.T)
    mask = np.triu(np.ones((P, P), np.float32))  # mask[s,t] = 1 if t>=s
    eye = np.eye(P, dtype=np.float32)

    in_maps = []
    for c in range(N_CORES):
        qs = slice(P * c, P * (c + 1))
        ks = slice(C + P * c, C + P * (c + 1))
        vs = slice(2 * C + P * c, 2 * C + P * (c + 1))
        # column order in wT: k, q, v (q-rows pre-scaled)
        w_c = np.concatenate(
            [wqkv_w[ks], wqkv_w[qs] * scale, wqkv_w[vs]], axis=0)  # [384, 1024]
        b_c = np.concatenate(
            [wqkv_b[ks], wqkv_b[qs] * scale, wqkv_b[vs]])          # [384]
        in_maps.append({
            "xT": xT,
            "wT": np.ascontiguousarray(w_c.T),
            "bqkv": np.ascontiguousarray(b_c.reshape(3, P).T),
            "pwT": np.ascontiguousarray(proj_w[qs].T),
            "pb": np.ascontiguousarray(proj_b[qs].reshape(P, 1)),
            "mask01": mask,
            "onesd": np.ones((P, D), np.float32),
            "ident": eye,
        })
    return in_maps


def kernel(x, wqkv_w, wqkv_b, proj_w, proj_b):
    global LAST_RESULT
    nc = _get_nc()
    in_maps = _prep_inputs(x, wqkv_w, wqkv_b, proj_w, proj_b)
    res = bass_utils.run_bass_kernel_spmd(nc, in_maps,
                                          core_ids=list(range(N_CORES)))
    LAST_RESULT = res
    full_outT = np.concatenate(
        [res.results[c]["outT"] for c in range(N_CORES)], axis=0)  # [1024, 2048]
    return np.ascontiguousarray(full_outT.T).astype(np.float32)


# revision 17
# speedup vs baseline: 1.8695x; 1.8695x over previous
"""Causal self-attention (T=2048, C=1024, H=16) on 8 Trainium2 NeuronCores.

Tensor-parallel over heads: each core owns 2 heads (wqkv row-shard), computes
qkv + attention for its heads, all-gathers the per-head attention outputs, then
computes its 128-column slice of the output projection (proj column-shard).

Structure: one software-pipelined loop over 4 token chunks of 512. For chunk g:
qkv(g) -> attention(g, both heads interleaved) -> AllGather(g) -> proj(g).
Later chunks' compute overlaps earlier chunks' collectives.

Layout notes (per core c, heads 2c and 2c+1):
  - xT   [1024, 2048]  x transposed (shared by all cores)
  - wT   [1024, 384]   wqkv rows for (k,q,v) of this core's heads, transposed;
                       q-rows pre-scaled by 1/sqrt(64)=0.125 (exact)
  - qT/kT/vT [128, 2048] in SBUF: rows = 2 heads x 64 dims, cols = tokens
  - scoresT  [s, t] tiles computed directly (no probs transpose needed);
    softmax denominator comes from an appended ones-column on v (row 64 of the
    attnT psum accumulator), so no cross-partition reductions are needed.
  - exp() is safe without max-subtraction: |scores| < 4 for this problem.

All matmul operands use float32r (single-pass PE, 4x the fp32 rate); set
KERNEL_FP32_EXACT=1 to fall back to exact fp32 matmuls.
"""

import os
import numpy as np

os.environ.setdefault("NEURON_RT_DBG_RDH_CC", "0")  # prefer mesh collectives

import concourse.bass as bass
import concourse.mybir as mybir
import concourse.tile as tile
from concourse import bacc
from concourse import bass_utils

T = 2048
C = 1024
H = 16
D = 64
N_CORES = 8
P = 128
NT = T // P          # 16 token tiles
NG = T // 512        # 4 column chunks of 512
NO = C // P          # 8 contraction subtiles

F32 = mybir.dt.float32
FAST_MM = os.environ.get("KERNEL_FP32_EXACT", "0") != "1"
MM = mybir.dt.float32r if FAST_MM else mybir.dt.float32


def _build():
    nc = bacc.Bacc("TRN2", target_bir_lowering=False, debug=False,
                   num_devices=N_CORES)

    xT = nc.dram_tensor("xT", [C, T], MM, kind="ExternalInput").ap()
    wT = nc.dram_tensor("wT", [C, 3 * P], MM, kind="ExternalInput").ap()
    bqkv = nc.dram_tensor("bqkv", [P, 3], F32, kind="ExternalInput").ap()
    pwT = nc.dram_tensor("pwT", [C, P], MM, kind="ExternalInput").ap()
    pb = nc.dram_tensor("pb", [P, 1], F32, kind="ExternalInput").ap()
    mask01 = nc.dram_tensor("mask01", [P, P], MM, kind="ExternalInput").ap()
    onesd = nc.dram_tensor("onesd", [P, D], MM, kind="ExternalInput").ap()
    ident = nc.dram_tensor("ident", [P, P], MM, kind="ExternalInput").ap()
    outT = nc.dram_tensor("outT", [P, T], F32, kind="ExternalOutput").ap()

    xT3 = xT.rearrange("(o p) t -> p o t", p=P)      # [128, 8, 2048]
    wT3 = wT.rearrange("(o p) j -> p o j", p=P)      # [128, 8, 384]
    pwT3 = pwT.rearrange("(o p) j -> p o j", p=P)    # [128, 8, 128]

    # wqkv section order in wT columns: k, q, v (k first so attention's
    # score matmuls can start earliest in the pipelined schedule)
    JK, JQ, JV = 0, 1, 2

    with tile.TileContext(nc) as tc:
        with (
            tc.tile_pool(name="const", bufs=1) as constp,
            tc.tile_pool(name="big", bufs=1) as bigp,
            tc.tile_pool(name="work", bufs=4) as workp,
            tc.tile_pool(name="projw", bufs=2) as projwp,
            tc.tile_pool(name="outp", bufs=3) as outp,
            tc.tile_pool(name="ps_big", bufs=2, space="PSUM") as ps_big,
            tc.tile_pool(name="ps_at", bufs=2, space="PSUM") as ps_at,
            tc.tile_pool(name="ps_sm", bufs=1, space="PSUM") as ps_sm,
            tc.tile_pool(name="ps_proj", bufs=1, space="PSUM") as ps_proj,
            tc.tile_pool(name="dram", bufs=1, space="DRAM") as dram,
        ):
            # ---- inputs: x and wqkv weights first (QKV needs them now) ----
            x_sb = bigp.tile([P, NO, T], MM, name="x")
            wT_sb = constp.tile([P, NO, 3 * P], MM, name="wT")
            for o in range(NO):
                nc.sync.dma_start(wT_sb[:, o, :], wT3[:, o, :])
                nc.sync.dma_start(x_sb[:, o, :], xT3[:, o, :])
            bq_sb = constp.tile([P, 3], F32, name="bqkv")
            nc.sync.dma_start(bq_sb[:], bqkv)
            id_sb = constp.tile([P, P], MM, name="ident")
            nc.sync.dma_start(id_sb[:], ident)
            mask_sb = constp.tile([P, P], MM, name="mask")
            nc.sync.dma_start(mask_sb[:], mask01)
            onesv_sb = constp.tile([P, NT], MM, name="onesv")
            nc.sync.dma_start(onesv_sb[:], onesd[:, 0:NT])
            ones_sb = constp.tile([1, D], F32, name="ones")
            nc.vector.memset(ones_sb[:], 1.0)
            pwT_sb = constp.tile([P, NO, P], MM, name="pwT")
            pb_sb = constp.tile([P, 1], F32, name="pb")

            qkvT = [bigp.tile([P, T], MM, name=n) for n in ("kT", "qT", "vT")]
            kT_sb, qT_sb, vT_sb = qkvT
            v_sb = bigp.tile([P, NT, 130], MM, name="v")
            nc.vector.tensor_copy(out=v_sb[:, :, D:D + 1],
                                  in_=onesv_sb[:, :, None])
            nc.vector.tensor_copy(out=v_sb[:, :, 2 * D + 1:2 * D + 2],
                                  in_=onesv_sb[:, :, None])

            warm_in = dram.tile([1, 4], MM, name="warm_in")
            warm_out = dram.tile([N_CORES, 4], MM, addr_space="Shared",
                                 name="warm_out")
            nc.sync.dma_start(warm_in[:], onesd[0:1, 0:4])
            nc.gpsimd.collective_compute(
                "AllGather",
                mybir.AluOpType.bypass,
                replica_groups=[list(range(N_CORES))],
                ins=[warm_in.opt()],
                outs=[warm_out.opt()],
            )

            ag_ins = [dram.tile([P, 512], MM, name=f"agi{g}") for g in range(NG)]
            ag_outs = [dram.tile([C, 512], MM, addr_space="Shared",
                                 name=f"ago{g}") for g in range(NG)]

            for g in range(NG):
                cols = slice(g * 512, (g + 1) * 512)

                # ---- qkv for this token chunk (k, q, then v) ----
                for j in (JK, JQ, JV):
                    ps = ps_big.tile([P, 1024], F32, name="sc2")
                    for o in range(NO):
                        nc.tensor.matmul(
                            ps[:, 0:512],
                            lhsT=wT_sb[:, o, j * P:(j + 1) * P],
                            rhs=x_sb[:, o, cols],
                            start=(o == 0), stop=(o == NO - 1),
                        )
                    # bias-add + copy on DVE (ACT stays free for exp)
                    nc.vector.tensor_scalar_add(qkvT[j][:, cols], ps[:, 0:512],
                                                bq_sb[:, j:j + 1])

                # v -> [s, d] tiles (with the ones column per head)
                for st in range(4 * g, 4 * g + 4):
                    pst = ps_sm.tile([P, 512], MM, name="sm")
                    nc.tensor.transpose(pst[:, 0:P],
                                        vT_sb[:, st * P:(st + 1) * P], id_sb[:])
                    nc.vector.tensor_copy(out=v_sb[:, st, 0:D],
                                          in_=pst[:, 0:D])
                    nc.vector.tensor_copy(out=v_sb[:, st, D + 1:2 * D + 1],
                                          in_=pst[:, D:2 * D])

                # ---- attention for chunk g, both heads interleaved ----
                ats = [ps_at.tile([P, 512], F32, name="at") for _ in range(2)]
                for j in range(4 * g + 4):
                    t0 = 512 * g if j < 4 * g else P * j
                    w_ = 512 * (g + 1) - t0
                    sc2 = ps_big.tile([P, 1024], F32, name="sc2")
                    for h in range(2):
                        nc.tensor.matmul(
                            sc2[:, 512 * h:512 * h + w_],
                            lhsT=kT_sb[h * D:(h + 1) * D, j * P:(j + 1) * P],
                            rhs=qT_sb[h * D:(h + 1) * D, t0:t0 + w_],
                            start=True, stop=True,
                        )
                    e2 = workp.tile([P, 1024], MM, name="e2")
                    if w_ == 512:
                        nc.scalar.activation(e2[:], sc2[:],
                                             mybir.ActivationFunctionType.Exp)
                    else:
                        for h in range(2):
                            nc.scalar.activation(
                                e2[:, 512 * h:512 * h + w_],
                                sc2[:, 512 * h:512 * h + w_],
                                mybir.ActivationFunctionType.Exp)
                    if j >= 4 * g:
                        # zero the strictly-upper (t<s) part of the diag block
                        for h in range(2):
                            nc.vector.tensor_mul(
                                out=e2[:, 512 * h:512 * h + P],
                                in0=e2[:, 512 * h:512 * h + P],
                                in1=mask_sb[:])
                    for h in range(2):
                        nc.tensor.matmul(
                            ats[h][:D + 1, t0 - 512 * g:512],
                            lhsT=v_sb[:, j, h * (D + 1):(h + 1) * (D + 1)],
                            rhs=e2[:, 512 * h:512 * h + w_],
                            start=(j == 0), stop=(j == 4 * g + 3),
                        )
                for h in range(2):
                    at = ats[h]
                    # rows 0..63 = unnormalized attnT, row 64 = softmax denom
                    rs = workp.tile([1, 512], F32, name="rs")
                    nc.vector.tensor_copy(out=rs[:], in_=at[D:D + 1, :])
                    rb = ps_sm.tile([P, 512], F32, name="sm")
                    nc.tensor.matmul(rb[:D, :], lhsT=ones_sb[:], rhs=rs[:],
                                     start=True, stop=True)
                    rr = workp.tile([D, 512], F32, name="rr")
                    nc.vector.reciprocal(rr[:], rb[:D, :])
                    ao = outp.tile([D, 512], MM, name="ao")
                    nc.vector.tensor_mul(out=ao[:], in0=at[0:D, :], in1=rr[:])
                    nc.sync.dma_start(ag_ins[g][h * D:(h + 1) * D, :], ao[:])

                nc.gpsimd.collective_compute(
                    "AllGather",
                    mybir.AluOpType.bypass,
                    replica_groups=[list(range(N_CORES))],
                    ins=[ag_ins[g].opt()],
                    outs=[ag_outs[g].opt()],
                )

                if g == 0:
                    # projection weights: needed from the first proj chunk on
                    nc.sync.dma_start(pwT_sb[:], pwT3)
                    nc.sync.dma_start(pb_sb[:], pb)

                # ---- projection for chunk g ----
                ag3 = ag_outs[g][:].rearrange("(o p) t -> p o t", p=P)
                r_sb = projwp.tile([P, NO, 512], MM, name="agr")
                nc.sync.dma_start(r_sb[:], ag3[:])
                psp = ps_proj.tile([P, 512], F32, name="proj")
                for o in range(NO):
                    nc.tensor.matmul(psp[:], lhsT=pwT_sb[:, o, :],
                                     rhs=r_sb[:, o, :],
                                     start=(o == 0), stop=(o == NO - 1))
                ob = outp.tile([P, 512], F32, name="ob")
                nc.scalar.activation(ob[:], psp[:],
                                     mybir.ActivationFunctionType.Identity,
                                     bias=pb_sb[:])
                nc.sync.dma_start(outT[:, cols], ob[:])

    nc.compile()
    return nc


_NC = None
LAST_RESULT = None


def _get_nc():
    global _NC
    if _NC is None:
        _NC = _build()
    return _NC


def _prep_inputs(x, wqkv_w, wqkv_b, proj_w, proj_b):
    x = np.asarray(x, np.float32)
    wqkv_w = np.asarray(wqkv_w, np.float32)
    wqkv_b = np.asarray(wqkv_b, np.float32)
    proj_w = np.asarray(proj_w, np.float32)
    proj_b = np.asarray(proj_b, np.float32)

    scale = np.float32(1.0 / np.sqrt(D))  # 0.125 exactly
    xT = np.ascontiguousarray(x.T)
    mask = np.triu(np.ones((P, P), np.float32))  # mask[s,t] = 1 if t>=s
    eye = np.eye(P, dtype=np.float32)

    in_maps = []
    for c in range(N_CORES):
        qs = slice(P * c, P * (c + 1))
        ks = slice(C + P * c, C + P * (c + 1))
        vs = slice(2 * C + P * c, 2 * C + P * (c + 1))
        # column order in wT: k, q, v (q-rows pre-scaled)
        w_c = np.concatenate(
            [wqkv_w[ks], wqkv_w[qs] * scale, wqkv_w[vs]], axis=0)  # [384, 1024]
        b_c = np.concatenate(
            [wqkv_b[ks], wqkv_b[qs] * scale, wqkv_b[vs]])          # [384]
        in_maps.append({
            "xT": xT,
            "wT": np.ascontiguousarray(w_c.T),
            "bqkv": np.ascontiguousarray(b_c.reshape(3, P).T),
            "pwT": np.ascontiguousarray(proj_w[qs].T),
            "pb": np.ascontiguousarray(proj_b[qs].reshape(P, 1)),
            "mask01": mask,
            "onesd": np.ones((P, D), np.float32),
            "ident": eye,
        })
    return in_maps


def kernel(x, wqkv_w, wqkv_b, proj_w, proj_b):
    global LAST_RESULT
    nc = _get_nc()
    in_maps = _prep_inputs(x, wqkv_w, wqkv_b, proj_w, proj_b)
    res = bass_utils.run_bass_kernel_spmd(nc, in_maps,
                                          core_ids=list(range(N_CORES)))
    LAST_RESULT = res
    full_outT = np.concatenate(
        [res.results[c]["outT"] for c in range(N_CORES)], axis=0)  # [1024, 2048]
    return np.ascontiguousarray(full_outT.T).astype(np.float32)
